# revision 6
# baseline (speedup 1.0000x reference)
"""Trainium2 Bass kernel for nn_KVCacheMoE (B=8, S=2048, H=1024, E=8).

Strategy: batch-parallel across the 8 NeuronCores (core c owns batch c).
The router depends only on that batch's tokens, so every core computes its
own routing weights locally and its full output shard — no collectives.

v3 design:
  - x path: fp32 loads on the sync HWDGE ring, ACT cast -> bf16, xbar
    DMA-transpose (scalar HWDGE ring, one call per tile; mapping
    xT[p, j, t] = x[t, j*128+p]).  Zero PE transposes.
  - We/Wr1 paths: fp32 raw tiles on HWDGE rings with ACT casts emitted
    one step behind each load (keeps ring FIFOs from blocking on
    buffer-reuse waits).  Expert-0 weights split across both rings so the
    first matmul starts at ~8us; PE warmup matmuls cover the prefix.
  - gpsimd: only xsum accumulation (tensor_tensor add) + be broadcasts.
  - Router: xmean via 8 ones-column matmuls, layer 1 as 64 bf16 LDW+MM
    pairs, layer 2 via 8 tiny MMs; expert-1 MM groups interleaved
    between router stages so the PE never idles and HAM stays at 8/8.
  - Epilogue per [128,1024] tile on DVE: stt mt=r*psum+r*be (->bf16),
    stt acc+=relu(mt) (bf16).  acc in bf16; expert 7 writes fp32 + DMA,
    final tile split in halves to shorten the tail.
"""
import numpy as np
from contextlib import ExitStack

import concourse.bass as bass
import concourse.tile as tile
from concourse import bacc, mybir
from concourse.bass_utils import run_bass_kernel_spmd

B, S, H, E = 8, 2048, 1024, 8
N_CORES = 8
P = 128
NF = 512
F32 = mybir.dt.float32
BF16 = mybir.dt.bfloat16
AX = mybir.AxisListType
ALU = mybir.AluOpType
ACTF = mybir.ActivationFunctionType

HJ = H // P           # 8
N_WARM = 30


def build_nc(s=S):
    t_tiles = s // P

    nc = bacc.Bacc("TRN2", target_bir_lowering=False, debug=False)
    x_ap = nc.dram_tensor("x", [s, H], F32, kind="ExternalInput").ap()
    we_ap = nc.dram_tensor("We", [E, H, H], F32, kind="ExternalInput").ap()
    be_ap = nc.dram_tensor("be", [E, H], F32, kind="ExternalInput").ap()
    wr1_ap = nc.dram_tensor("Wr1", [H, H], F32, kind="ExternalInput").ap()
    br1_ap = nc.dram_tensor("br1", [H], F32, kind="ExternalInput").ap()
    wr2_ap = nc.dram_tensor("Wr2", [H, E], F32, kind="ExternalInput").ap()
    br2_ap = nc.dram_tensor("br2", [E], F32, kind="ExternalInput").ap()
    out_ap = nc.dram_tensor("out", [s, H], F32, kind="ExternalOutput").ap()

    with tile.TileContext(nc) as tc, ExitStack() as ctx:
        xsp = ctx.enter_context(tc.tile_pool(name="xsp", bufs=3))
        xbp = ctx.enter_context(tc.tile_pool(name="xbp", bufs=6))
        xtpool = ctx.enter_context(tc.tile_pool(name="xt", bufs=1))
        accpool = ctx.enter_context(tc.tile_pool(name="acc", bufs=1))
        wqpool = ctx.enter_context(tc.tile_pool(name="wq", bufs=2))
        wraw = ctx.enter_context(tc.tile_pool(name="wraw", bufs=3))
        wr1rawp = ctx.enter_context(tc.tile_pool(name="w1raw", bufs=2))
        wrpool = ctx.enter_context(tc.tile_pool(name="wrp", bufs=1))
        bepool = ctx.enter_context(tc.tile_pool(name="bep", bufs=2))
        mtpool = ctx.enter_context(tc.tile_pool(name="mt", bufs=3))
        outpool = ctx.enter_context(tc.tile_pool(name="ob", bufs=2))
        rpool = ctx.enter_context(tc.tile_pool(name="rp", bufs=1))
        ps = ctx.enter_context(tc.tile_pool(name="ps", bufs=3, space="PSUM"))
        psr = ctx.enter_context(tc.tile_pool(name="psr", bufs=2, space="PSUM"))

        # ---- constants / scratch ----
        ones_col = rpool.tile([P, 1], BF16, tag="ones_col")
        nc.vector.memset(ones_col, 1.0)
        ones_row = rpool.tile([1, P], F32, tag="ones_row")
        nc.vector.memset(ones_row, 1.0)
        wsrc = rpool.tile([P, 128 + NF], BF16, tag="wsrc")
        nc.vector.memset(wsrc, 0.5)

        # persistent residents
        xT = xtpool.tile([P, HJ, s], BF16, tag="xT", name="xT")
        acc = [accpool.tile([P, H], BF16, tag=f"acc{i}", name=f"acc{i}")
               for i in range(t_tiles)]
        accx = rpool.tile([P, H], F32, tag="accx")
        wr1b = wrpool.tile([P, HJ, H], BF16, tag="wr1b", name="wr1b")
        w2b = rpool.tile([P, HJ, E], BF16, tag="w2b")
        br1t = rpool.tile([P, HJ], F32, tag="br1t")
        br2t = rpool.tile([1, E], F32, tag="br2t")

        # ---- PE warmup (garbage matmuls; cover the weight-load prefix) ----
        warm_ps = psr.tile([P, NF], F32, tag="psr", name="warm")
        for _ in range(N_WARM):
            nc.tensor.matmul(warm_ps[:], wsrc[:, 0:P], wsrc[:, P:P + NF],
                             start=True, stop=True)

        # ---- expert-0 weights: fp32 split across both HWDGE rings ----
        wq0 = wqpool.tile([P, HJ, H], BF16, tag="wq", name="wq0")
        w0r = []
        for hj in range(HJ):
            wr = wraw.tile([P, H], F32, tag="wr", name=f"w0r{hj}")
            eng = nc.sync if hj < 4 else nc.scalar
            eng.dma_start(wr[:], we_ap[0, bass.ts(hj, P), :])
            w0r.append(wr)
            if hj >= 2:
                # cast two steps behind the loads (slots free before reuse)
                nc.scalar.copy(wq0[:, hj - 2, :], w0r[hj - 2][:])
        nc.scalar.copy(wq0[:, 6, :], w0r[6][:])
        nc.scalar.copy(wq0[:, 7, :], w0r[7][:])
        # router biases + Wr2 on the scalar ring
        nc.scalar.dma_start(br1t[:], br1_ap.rearrange("(j p) -> p j", p=P))
        nc.scalar.dma_start(br2t[:], br2_ap.rearrange("(a e) -> a e", a=1))
        w2raw = rpool.tile([P, HJ, E], F32, tag="w2raw")
        nc.scalar.dma_start(w2raw[:], wr2_ap.rearrange("(j p) e -> p j e", p=P))

        # expert-0 bias broadcast (gpsimd DRE)
        ber0 = bepool.tile([P, H], F32, tag="ber", name="ber0")
        nc.gpsimd.dma_start(ber0[:], be_ap[0:1, :].to_broadcast([P, H]))
        nc.gpsimd.memset(accx[:], 0.0)

        # ---- phase A: x stream + transpose + xsum + expert 0 ----
        xs_tiles = {}
        xb_tiles = {}

        def load_xs(ti):
            t = xsp.tile([P, H], F32, tag="xs", name=f"xs{ti}")
            nc.sync.dma_start(t[:], x_ap[bass.ts(ti, P), :])
            xs_tiles[ti] = t

        def cast_transpose(ti):
            xb = xbp.tile([P, H], BF16, tag="xb", name=f"xb{ti}")
            nc.scalar.copy(xb[:], xs_tiles[ti][:])
            nc.scalar.dma_start(xT[:, :, bass.ts(ti, P)], xb[:], transpose=True)
            xb_tiles[ti] = xb

        load_xs(0)
        load_xs(1)
        cast_transpose(0)

        wq1 = wqpool.tile([P, HJ, H], BF16, tag="wq", name="wq1")
        wr1raw = [None] * HJ
        w1r = [None] * HJ

        for ti in range(t_tiles):
            if ti + 2 < t_tiles:
                load_xs(ti + 2)
            if ti + 1 < t_tiles:
                cast_transpose(ti + 1)
            # Wr1 raw: one block per ti at ti=4..11, cast one step behind
            if 4 <= ti < 12:
                k = ti - 4
                wr = wr1rawp.tile([P, H], F32, tag="wr1", name=f"wr1r{k}")
                nc.scalar.dma_start(wr[:], wr1_ap[bass.ts(k, P), :])
                wr1raw[k] = wr
            if 5 <= ti < 13:
                k = ti - 5
                nc.scalar.copy(wr1b[:, k, :], wr1raw[k][:])
            # expert-1 raw: one block per ti at ti=8..15, cast one behind
            if 8 <= ti < 16:
                k = ti - 8
                wr = wraw.tile([P, H], F32, tag="wr", name=f"w1r{k}")
                nc.scalar.dma_start(wr[:], we_ap[1, bass.ts(k, P), :])
                w1r[k] = wr
            if 9 <= ti < 16:
                k = ti - 9
                nc.scalar.copy(wq1[:, k, :], w1r[k][:])
            # xsum accumulation on gpsimd
            nc.gpsimd.tensor_tensor(accx[:], accx[:], xb_tiles[ti][:], op=ALU.add)

            # expert-0 matmuls
            mm_ps = ps.tile([P, H], F32, tag="ps")
            for dc in range(2):
                for hj in range(HJ):
                    nc.tensor.matmul(
                        mm_ps[:, bass.ts(dc, NF)],
                        xT[:, hj, bass.ts(ti, P)],
                        wq0[:, hj, bass.ts(dc, NF)],
                        start=(hj == 0),
                        stop=(hj == HJ - 1),
                    )
            # unscaled epilogue: acc = relu(psum + be0); scaled by r0 at e1
            mt = mtpool.tile([P, H], BF16, tag="mt")
            nc.vector.tensor_tensor(mt[:], mm_ps[:], ber0[:], op=ALU.add)
            nc.vector.tensor_scalar_max(acc[ti][:], mt[:], 0.0)

        nc.scalar.copy(wq1[:, 7, :], w1r[7][:])
        nc.scalar.copy(w2b[:], w2raw[:])

        # ---- router (expert-1 MM groups interleaved to keep PE busy) ----
        axb = rpool.tile([P, H], BF16, tag="axb")
        nc.scalar.copy(axb[:], accx[:])

        e1_ps = []

        def e1_group(ti):
            g = ps.tile([P, H], F32, tag="ps")
            for dc in range(2):
                for hj in range(HJ):
                    nc.tensor.matmul(
                        g[:, bass.ts(dc, NF)],
                        xT[:, hj, bass.ts(ti, P)],
                        wq1[:, hj, bass.ts(dc, NF)],
                        start=(hj == 0),
                        stop=(hj == HJ - 1),
                    )
            e1_ps.append(g)

        xm_ps = psr.tile([P, HJ], F32, tag="psr", name="xmps")
        for j in range(HJ):
            nc.tensor.matmul(xm_ps[:, j:j + 1], axb[:, bass.ts(j, P)],
                             ones_col[:], start=True, stop=True)
        xmean = rpool.tile([P, HJ], BF16, tag="xmean")
        nc.scalar.mul(xmean[:], xm_ps[:], 1.0 / s)

        e1_group(0)

        hv_ps = psr.tile([P, HJ], F32, tag="psr", name="hvps")
        for dj in range(HJ):
            for hj in range(HJ):
                nc.tensor.matmul(
                    hv_ps[:, dj:dj + 1],
                    wr1b[:, hj, bass.ts(dj, P)],
                    xmean[:, hj:hj + 1],
                    start=(hj == 0),
                    stop=(hj == HJ - 1),
                )
        hsb = rpool.tile([P, HJ], BF16, tag="hsb")
        hs1 = rpool.tile([P, HJ], F32, tag="hs1")
        nc.vector.tensor_tensor(hs1[:], hv_ps[:], br1t[:], op=ALU.add)
        nc.vector.tensor_scalar_max(hsb[:], hs1[:], 0.0)

        e1_group(1)

        lg_ps = psr.tile([1, E], F32, tag="psr", name="lgps")
        for dj in range(HJ):
            nc.tensor.matmul(lg_ps[:], hsb[:, dj:dj + 1], w2b[:, dj, :],
                             start=(dj == 0), stop=(dj == HJ - 1))
        logits = rpool.tile([1, E], F32, tag="logits")
        nc.vector.tensor_tensor(logits[:], lg_ps[:], br2t[:], op=ALU.add)
        mx = rpool.tile([1, 1], F32, tag="mx")
        nc.vector.reduce_max(mx[:], logits[:], axis=AX.X)
        nmx = rpool.tile([1, 1], F32, tag="nmx")
        nc.vector.tensor_scalar_mul(nmx[:], mx[:], -1.0)
        ex = rpool.tile([1, E], F32, tag="ex")
        nc.scalar.activation(ex[:], logits[:], ACTF.Exp, bias=nmx[:], scale=1.0)
        sm = rpool.tile([1, 1], F32, tag="sm")
        nc.vector.reduce_sum(sm[:], ex[:], axis=AX.X)
        rinv = rpool.tile([1, 1], F32, tag="rinv")
        nc.vector.reciprocal(rinv[:], sm[:])
        rvec = rpool.tile([1, E], F32, tag="rvec")
        nc.vector.tensor_scalar_mul(rvec[:], ex[:], rinv[:])

        e1_group(2)

        rsb_ps = psr.tile([P, E], F32, tag="psr", name="rsbps")
        nc.tensor.matmul(rsb_ps[:], ones_row[:], rvec[:], start=True, stop=True)
        rsb = rpool.tile([P, E], F32, tag="rsb")
        nc.scalar.copy(rsb[:], rsb_ps[:])

        # ---- experts 1..7 ----
        wq = wq1
        for e in range(1, E):
            ber = bepool.tile([P, H], F32, tag="ber")
            nc.gpsimd.dma_start(ber[:], be_ap[e:e + 1, :].to_broadcast([P, H]))
            bep = bepool.tile([P, H], F32, tag="bep")
            nc.scalar.mul(bep[:], ber[:], rsb[:, e:e + 1])

            if e < E - 1:
                wq_next = wqpool.tile([P, HJ, H], BF16, tag="wq",
                                      name=f"wq{e + 1}")
            wnr = [None] * HJ
            for ti in range(t_tiles):
                if e == 1:
                    # deferred expert-0 routing weight (ACT, off the DVE path)
                    nc.scalar.mul(acc[ti][:], acc[ti][:], rsb[:, 0:1])
                if e == 1 and ti < 3:
                    mm_ps = e1_ps[ti]
                else:
                    mm_ps = ps.tile([P, H], F32, tag="ps")
                    for dc in range(2):
                        for hj in range(HJ):
                            nc.tensor.matmul(
                                mm_ps[:, bass.ts(dc, NF)],
                                xT[:, hj, bass.ts(ti, P)],
                                wq[:, hj, bass.ts(dc, NF)],
                                start=(hj == 0),
                                stop=(hj == HJ - 1),
                            )
                if e < E - 1 and ti < HJ:
                    # next expert's raw weights + inline cast (one behind)
                    wr = wraw.tile([P, H], F32, tag="wr", name=f"w{e + 1}r{ti}")
                    nc.scalar.dma_start(wr[:], we_ap[e + 1, bass.ts(ti, P), :])
                    wnr[ti] = wr
                if e < E - 1 and 1 <= ti <= HJ:
                    k = ti - 1
                    nc.scalar.copy(wq_next[:, k, :], wnr[k][:])

                last_tile = (e == E - 1 and ti == t_tiles - 1)
                if not last_tile:
                    mt = mtpool.tile([P, H], BF16, tag="mt")
                    nc.vector.scalar_tensor_tensor(
                        mt[:], mm_ps[:], rsb[:, e:e + 1], bep[:],
                        op0=ALU.mult, op1=ALU.add,
                    )
                    if e < E - 1:
                        nc.vector.scalar_tensor_tensor(
                            acc[ti][:], mt[:], 0.0, acc[ti][:],
                            op0=ALU.max, op1=ALU.add,
                        )
                    else:
                        obuf = outpool.tile([P, H], F32, tag="ob")
                        nc.vector.scalar_tensor_tensor(
                            obuf[:], mt[:], 0.0, acc[ti][:],
                            op0=ALU.max, op1=ALU.add,
                        )
                        nc.sync.dma_start(out_ap[bass.ts(ti, P), :], obuf[:])
                else:
                    # split the final tile in halves to shorten the tail
                    obuf = outpool.tile([P, H], F32, tag="ob")
                    for half in range(2):
                        hsl = bass.ts(half, NF)
                        mth = mtpool.tile([P, NF], BF16, tag="mth")
                        nc.vector.scalar_tensor_tensor(
                            mth[:], mm_ps[:, hsl], rsb[:, e:e + 1], bep[:, hsl],
                            op0=ALU.mult, op1=ALU.add,
                        )
                        nc.vector.scalar_tensor_tensor(
                            obuf[:, hsl], mth[:], 0.0, acc[ti][:, hsl],
                            op0=ALU.max, op1=ALU.add,
                        )
                        nc.sync.dma_start(out_ap[bass.ts(ti, P), hsl],
                                          obuf[:, hsl])
            if e < E - 1:
                wq = wq_next

    nc.compile()
    return nc


_nc_cache = {}


def _get_nc(s):
    if s not in _nc_cache:
        _nc_cache[s] = build_nc(s)
    return _nc_cache[s]


def kernel(x, We, be, Wr1, br1, Wr2, br2):
    x = np.ascontiguousarray(np.asarray(x, dtype=np.float32))
    We = np.ascontiguousarray(np.asarray(We, dtype=np.float32))
    be = np.ascontiguousarray(np.asarray(be, dtype=np.float32))
    Wr1 = np.ascontiguousarray(np.asarray(Wr1, dtype=np.float32))
    br1 = np.ascontiguousarray(np.asarray(br1, dtype=np.float32))
    Wr2 = np.ascontiguousarray(np.asarray(Wr2, dtype=np.float32))
    br2 = np.ascontiguousarray(np.asarray(br2, dtype=np.float32))

    s = x.shape[1]
    nc = _get_nc(s)
    shared = {"We": We, "be": be, "Wr1": Wr1, "br1": br1, "Wr2": Wr2, "br2": br2}
    in_maps = [{"x": x[c], **shared} for c in range(N_CORES)]
    res = run_bass_kernel_spmd(nc, in_maps, list(range(N_CORES)))
    return np.stack([res.results[c]["out"] for c in range(N_CORES)], axis=0)
